# revision 17
# baseline (speedup 1.0000x reference)
# Trainium2 Bass kernel for nn_AttentionBlock (additive attention over S):
#   pre = W1h @ hid (bcast over S) + W1e @ enc + b1      [B,S,A,HW]
#   energy[b,s] = mean_hw( W2 . tanh(pre) )              (+b2 dropped: softmax
#   attn = softmax_S(energy); ctx = sum_s attn[s]*enc     shift-invariant)
#
# Sharding: data-parallel over batch. B=16 over 8 cores -> 2 batches/core.
#
# v2 design (vs the 70.7us f32r baseline):
#  - energy path in fp8-e4m3 with DoubleRow matmuls: one instruction contracts
#    both 128-deep c-chunks at 0.5 cycles/row -> 4x less PE time than f32r.
#    The eh term is injected as a second DoubleRow matmul of W1h @ hid
#    (hid duplicated per s-pair in SBUF), which also kills the separate
#    eh prologue and its Identity activation; b1 rides the tanh bias port.
#  - tanh on ScalarE in [128, 4*256] PSUM->SBUF bf16 tiles (quad of s).
#  - spatial sums via per-s DVE tensor_scalar(mult 1.0, accum_out=...) from
#    the bf16 tanh tile: SBUF-only 2-byte operands hit the 4x DVE mode
#    (tensor_reduce has no fast modes and would cost 2x more than tanh).
#  - context path in bf16: host-cast enc copy, scaled-identity matmuls with
#    bf16 sids (built by 4x-mode tensor_scalar from the broadcast attn).
#  - fp8/bf16 numerics validated against the fp32 reference in numpy:
#    rel err 3.3e-3 (threshold 2e-2); energies are tiny (|e| < 0.13) so exp
#    without max-subtraction is safe.
#
# Projected engine busy per rep (cost model): DMA ~37-39us (12.5 MiB staged
# at ~345 GB/s), ACT ~34, PE ~28, DVE ~25.

import numpy as np

B, S, C, A, HW = 16, 32, 256, 256, 256
NCORES = 8
BPC = B // NCORES  # batches per core


def build_program(reps: int = 1):
    import concourse.bacc as bacc
    import concourse.tile as tile
    from concourse import mybir
    from contextlib import ExitStack

    f32 = mybir.dt.float32
    f32r = mybir.dt.float32r
    bf16 = mybir.dt.bfloat16
    fp8 = mybir.dt.float8e4
    fp8e5 = mybir.dt.float8e5
    AF = mybir.ActivationFunctionType
    DR = mybir.MatmulPerfMode.DoubleRow
    ts = lambda i, n: slice(i * n, (i + 1) * n)

    nc = bacc.Bacc(
        "TRN2",
        target_bir_lowering=False,
        debug=False,
        enable_asserts=False,
        num_devices=NCORES,
    )

    # HBM tensors. enc8/encb/hid8 are host-packed as [b, p, kc, s, hw] with
    # c = kc*128 + p so each DMA lands directly in matmul-ready layout with
    # >=2KB contiguous runs per partition.
    enc8_d = nc.dram_tensor("enc8", [BPC, 128, 2, S, HW], fp8, kind="ExternalInput").ap()
    encr_d = nc.dram_tensor("encr", [BPC, 128, 2, S, HW], fp8, kind="ExternalInput").ap()
    hid8_d = nc.dram_tensor("hid8", [BPC, 128, 2, 2, HW], fp8, kind="ExternalInput").ap()
    w1e8_d = nc.dram_tensor("w1e8", [128, 2, A], fp8, kind="ExternalInput").ap()
    w1h8_d = nc.dram_tensor("w1h8", [128, 2, A], fp8, kind="ExternalInput").ap()
    b1_d = nc.dram_tensor("b1c", [128, A // 128], f32, kind="ExternalInput").ap()
    w2_d = nc.dram_tensor("w2c", [128, A // 128], f32r, kind="ExternalInput").ap()
    idb_d = nc.dram_tensor("identb", [128, 128], bf16, kind="ExternalInput").ap()
    ones_d = nc.dram_tensor("ones", [1, 128], f32r, kind="ExternalInput").ap()
    out_d = nc.dram_tensor("ctx", [BPC, C, HW], f32r, kind="ExternalOutput").ap()

    with tile.TileContext(nc) as tc, ExitStack() as ctx:
        consts = ctx.enter_context(tc.tile_pool(name="consts", bufs=1))
        encp8 = ctx.enter_context(tc.tile_pool(name="encp8", bufs=3))
        encpb = ctx.enter_context(tc.tile_pool(name="encpb", bufs=3))
        hidp = ctx.enter_context(tc.tile_pool(name="hidp", bufs=2))
        accp = ctx.enter_context(tc.tile_pool(name="accp", bufs=4))
        scrp = ctx.enter_context(tc.tile_pool(name="scrp", bufs=4))
        junkp = ctx.enter_context(tc.tile_pool(name="junkp", bufs=4))
        smp = ctx.enter_context(tc.tile_pool(name="smp", bufs=4))
        sidp = ctx.enter_context(tc.tile_pool(name="sidp", bufs=2))
        outp = ctx.enter_context(tc.tile_pool(name="outp", bufs=4))
        preps = ctx.enter_context(tc.tile_pool(name="preps", bufs=2, space="PSUM"))
        smallps = ctx.enter_context(tc.tile_pool(name="smallps", bufs=2, space="PSUM"))
        ctxps = ctx.enter_context(tc.tile_pool(name="ctxps", bufs=2, space="PSUM"))

        # constants
        w1e8_sb = consts.tile([128, 2, A], fp8, tag="w1e8_sb")
        nc.sync.dma_start(w1e8_sb[:], w1e8_d[:])
        w1h8_sb = consts.tile([128, 2, A], fp8, tag="w1h8_sb")
        nc.sync.dma_start(w1h8_sb[:], w1h8_d[:])
        b1_sb = consts.tile([128, 2], f32, tag="b1_sb")
        nc.sync.dma_start(b1_sb[:], b1_d[:])
        w2_sb = consts.tile([128, 2], f32r, tag="w2_sb")
        nc.sync.dma_start(w2_sb[:], w2_d[:])
        idb_sb = consts.tile([128, 128], bf16, tag="idb_sb")
        nc.sync.dma_start(idb_sb[:], idb_d[:])
        ones_sb = consts.tile([1, 128], f32r, tag="ones_sb")
        nc.sync.dma_start(ones_sb[:], ones_d[:])

        def phase_energy(b, hid8_sb, e8_t):
            """DR matmuls + tanh + 4x-mode accumulate; returns acc tiles."""
            acc = []
            for ach in range(2):
                a_t = accp.tile([128, S], f32r, tag="acc")
                acc.append(a_t)
            for ach in range(2):
                for quad in range(8):
                    pre = preps.tile([128, 4, HW], f32, tag="pre")
                    for half in range(2):
                        s0 = 4 * quad + 2 * half
                        nc.tensor.matmul(
                            pre[:, 2 * half : 2 * half + 2, :],
                            lhsT=w1e8_sb[:, :, ts(ach, 128)],
                            rhs=e8_t[:, :, s0 : s0 + 2, :],
                            start=True,
                            stop=False,
                            perf_mode=DR,
                        )
                        nc.tensor.matmul(
                            pre[:, 2 * half : 2 * half + 2, :],
                            lhsT=w1h8_sb[:, :, ts(ach, 128)],
                            rhs=hid8_sb[:],
                            start=False,
                            stop=True,
                            perf_mode=DR,
                        )
                    tq = scrp.tile([128, 4, HW], bf16, tag="tq")
                    nc.scalar.activation(
                        tq[:], pre[:], AF.Tanh, bias=b1_sb[:, ach : ach + 1]
                    )
                    with nc.allow_low_precision(reason="bf16 scratch, f32 accum"):
                        for si in range(4):
                            s = 4 * quad + si
                            scr = junkp.tile([128, HW], bf16, tag="scr")
                            nc.vector.tensor_scalar(
                                scr[:],
                                tq[:, si, :],
                                1.0,
                                0.0,
                                op0=mybir.AluOpType.mult,
                                op1=mybir.AluOpType.add,
                                accum_out=acc[ach][:, s : s + 1],
                            )
            return acc

        def phase_softmax(acc):
            """energies -> softmax -> fp8 sid pair (main + x16 residual)."""
            ep = smallps.tile([1, S], f32, tag="ep", bufs=1)
            for ach in range(2):
                nc.tensor.matmul(
                    ep[:],
                    lhsT=w2_sb[:, ach : ach + 1],
                    rhs=acc[ach][:],
                    start=(ach == 0),
                    stop=(ach == 1),
                )
            # softmax over S (|energy| < 0.2, exp safe without max-sub)
            esb = smp.tile([1, S], f32, tag="esb")
            nc.scalar.activation(esb[:], ep[:], AF.Exp)
            zsum = smp.tile([1, 1], f32, tag="zsum")
            nc.vector.reduce_sum(zsum[:], esb[:], axis=mybir.AxisListType.X)
            rz = smp.tile([1, 1], f32, tag="rz")
            nc.vector.reciprocal(rz[:], zsum[:])
            # ab2[0,0,:] = attn; ab2[0,1,:] = attn - fp8(attn). The residual
            # sids are stored e5m2 (normals reach 6e-5) so no rescale needed
            # and all three context passes share one PSUM accumulation group.
            ab2 = smp.tile([1, 2, S], f32r, tag="ab2")
            with nc.allow_low_precision(reason="fp8 sid chain, validated 2e-3"):
                nc.vector.tensor_scalar_mul(ab2[:, 0, :], esb[:], rz[:, 0:1])
                a8 = smp.tile([1, S], fp8, tag="a8")
                nc.vector.tensor_copy(a8[:], ab2[:, 0, :])
                nc.vector.scalar_tensor_tensor(
                    ab2[:, 1, :],
                    in0=a8[:],
                    scalar=-1.0,
                    in1=ab2[:, 0, :],
                    op0=mybir.AluOpType.mult,
                    op1=mybir.AluOpType.add,
                )
            # broadcast both rows to all 128 partitions with one K=1 matmul
            abc_ps = smallps.tile([128, 2, S], f32, tag="abc", bufs=1)
            nc.tensor.matmul(
                abc_ps[:], lhsT=ones_sb[:], rhs=ab2[:], start=True, stop=True
            )
            abc = smp.tile([128, 2, S], f32, tag="abc_sb")
            nc.vector.tensor_copy(abc[:], abc_ps[:])
            sid8 = sidp.tile([128, S, 128], fp8, tag="sid8")
            sidf8 = sidp.tile([128, S, 128], fp8e5, tag="sidf8")
            with nc.allow_low_precision(reason="fp8 sids feed fp8 DR matmul"):
                for s in range(S):
                    nc.vector.tensor_scalar_mul(
                        sid8[:, s, :], idb_sb[:], abc[:, 0, s : s + 1]
                    )
                for s in range(S):
                    nc.gpsimd.tensor_scalar_mul(
                        sidf8[:, s, :], idb_sb[:], abc[:, 1, s : s + 1]
                    )
            return sid8, sidf8

        def phase_context(b, sids, e8_t, er_t):
            """DoubleRow fp8 context: a8*(enc8+r8) + af5*enc8, one PSUM
            accumulation group per c-chunk."""
            sid8, sidf8 = sids
            ctxp = []
            for kc in range(2):
                cp_t = ctxps.tile([128, HW], f32, tag="ctxp")
                ctxp.append(cp_t)
            for p in range(S // 2):
                s0 = 2 * p
                for kc in range(2):
                    nc.tensor.matmul(
                        ctxp[kc][:],
                        lhsT=sid8[:, s0 : s0 + 2, :],
                        rhs=e8_t[:, kc, s0 : s0 + 2, :],
                        start=(p == 0),
                        stop=False,
                        perf_mode=DR,
                    )
            for p in range(S // 2):
                s0 = 2 * p
                for kc in range(2):
                    nc.tensor.matmul(
                        ctxp[kc][:],
                        lhsT=sid8[:, s0 : s0 + 2, :],
                        rhs=er_t[:, kc, s0 : s0 + 2, :],
                        start=False,
                        stop=False,
                        perf_mode=DR,
                    )
            for p in range(S // 2):
                s0 = 2 * p
                for kc in range(2):
                    nc.tensor.matmul(
                        ctxp[kc][:],
                        lhsT=sidf8[:, s0 : s0 + 2, :],
                        rhs=e8_t[:, kc, s0 : s0 + 2, :],
                        start=False,
                        stop=(p == S // 2 - 1),
                        perf_mode=DR,
                    )
            for kc in range(2):
                osb = outp.tile([128, HW], f32r, tag="osb")
                nc.vector.tensor_copy(osb[:], ctxp[kc][:])
                nc.sync.dma_start(out_d[b, ts(kc, 128), :], osb[:])

        for rep in range(reps):
            # hidden first: tiny DMAs must not queue behind encoder traffic
            hid8_sb, e8_sb, eb_sb = [], [], []
            for b in range(BPC):
                h_t = hidp.tile([128, 2, 2, HW], fp8, tag="hid8_sb")
                nc.sync.dma_start(h_t[:], hid8_d[b])
                hid8_sb.append(h_t)
            for b in range(BPC):
                # encoder tiles: fp8 main for the energy path (needed first)
                # plus fp8 residual for the context path (needed after
                # softmax); single whole-tile DMAs keep the SP sequencer cheap
                e8_t = encp8.tile([128, 2, S, HW], fp8, tag="enc8_sb")
                nc.sync.dma_start(e8_t[:], enc8_d[b])
                e8_sb.append(e8_t)
                er_t = encpb.tile([128, 2, S, HW], fp8, tag="encr_sb")
                nc.sync.dma_start(er_t[:], encr_d[b])
                eb_sb.append(er_t)

            # interleave the two batches so batch 1's energy matmuls keep PE
            # busy while batch 0's softmax tail resolves on ACT/DVE, and
            # batch 0's context hides batch 1's softmax tail
            acc0 = phase_energy(0, hid8_sb[0], e8_sb[0])
            sid0 = phase_softmax(acc0)
            acc1 = phase_energy(1, hid8_sb[1], e8_sb[1])
            sid1 = phase_softmax(acc1)
            phase_context(0, sid0, e8_sb[0], eb_sb[0])
            phase_context(1, sid1, e8_sb[1], eb_sb[1])

    nc.compile()
    return nc


def make_in_maps(hidden_state, encoder_outputs, W1, b1, W2):
    import ml_dtypes

    e4 = ml_dtypes.float8_e4m3
    bf = ml_dtypes.bfloat16

    hs = np.ascontiguousarray(hidden_state, dtype=np.float32).reshape(B, C, HW)
    enc = np.ascontiguousarray(encoder_outputs, dtype=np.float32).reshape(B, S, C, HW)
    # [b, s, (kc p), hw] -> [b, p, kc, s, hw]
    enc_pack = enc.reshape(B, S, 2, 128, HW).transpose(0, 3, 2, 1, 4)
    enc8 = np.ascontiguousarray(enc_pack.astype(e4))
    encr = np.ascontiguousarray(
        (enc_pack - enc8.astype(np.float32)).astype(e4)
    )
    # hid [b, (kc p), hw] -> [b, p, kc, hw], duplicated over the s-pair dim
    hid_pack = hs.reshape(B, 2, 128, HW).transpose(0, 2, 1, 3)
    hid8 = np.ascontiguousarray(
        np.broadcast_to(hid_pack[:, :, :, None, :], (B, 128, 2, 2, HW)).astype(e4)
    )
    w1 = np.asarray(W1, np.float32)
    # W1 is [a, c]; lhsT layout [c_part, kc, a]
    w1e8 = np.ascontiguousarray(w1[:, C:].T.reshape(2, 128, A).transpose(1, 0, 2).astype(e4))
    w1h8 = np.ascontiguousarray(w1[:, :C].T.reshape(2, 128, A).transpose(1, 0, 2).astype(e4))
    b1c = np.ascontiguousarray(np.asarray(b1, np.float32).reshape(2, 128).T)
    w2c = np.ascontiguousarray((np.asarray(W2, np.float32)[0] / HW).reshape(2, 128).T)
    identb = np.eye(128, dtype=np.float32).astype(bf)
    ones = np.ones((1, 128), dtype=np.float32)
    in_maps = []
    for i in range(NCORES):
        in_maps.append(
            {
                "enc8": enc8[BPC * i : BPC * (i + 1)],
                "encr": encr[BPC * i : BPC * (i + 1)],
                "hid8": hid8[BPC * i : BPC * (i + 1)],
                "w1e8": w1e8,
                "w1h8": w1h8,
                "b1c": b1c,
                "w2c": w2c,
                "identb": identb,
                "ones": ones,
            }
        )
    return in_maps


def _wait_devices_healthy(max_tries=20, sleep_s=20):
    import time
    import jax

    for i in range(max_tries):
        try:
            for d in jax.devices()[:NCORES]:
                np.asarray(jax.device_put(np.ones(4, np.float32), d) + 1)
            return
        except Exception:
            if i == max_tries - 1:
                raise
            time.sleep(sleep_s)


def kernel(hidden_state, encoder_outputs, W1, b1, W2, b2, _profile=[None]):
    import os
    import time

    # The axon NTFF-profiling hook is unavailable in this environment; make
    # sure run_bass_kernel_spmd never takes the trace path.
    os.environ["BASS_NEVER_TRACE"] = "1"
    from concourse.bass_utils import run_bass_kernel_spmd

    _wait_devices_healthy()
    nc = build_program()
    in_maps = make_in_maps(hidden_state, encoder_outputs, W1, b1, W2)
    res = None
    for attempt in range(3):
        try:
            res = run_bass_kernel_spmd(nc, in_maps, list(range(NCORES)))
            break
        except Exception:
            if attempt == 2:
                raise
            time.sleep(30)
            _wait_devices_healthy()
    _profile[0] = res
    out = np.empty((B, C, 16, 16), dtype=np.float32)
    for i in range(NCORES):
        out[BPC * i : BPC * (i + 1)] = res.results[i]["ctx"].reshape(BPC, C, 16, 16)
    return out


# revision 19
# speedup vs baseline: 2.1566x; 2.1566x over previous
# Trainium2 Bass kernel for nn_AttentionBlock (additive attention over S):
#   pre = W1h @ hid (bcast over S) + W1e @ enc + b1      [B,S,A,HW]
#   energy[b,s] = mean_hw( W2 . tanh(pre) )              (+b2 dropped: softmax
#   attn = softmax_S(energy); ctx = sum_s attn[s]*enc     shift-invariant)
#
# Sharding: data-parallel over batch. B=16 over 8 cores -> 2 batches/core.
#
# Design (vs the 70.7us f32r baseline):
#  - energy path in fp8-e4m3 with DoubleRow matmuls: one instruction contracts
#    both 128-deep c-chunks at 0.5 cycles/row -> 4x less PE time than f32r.
#    The eh term is injected as a second DoubleRow matmul of W1h @ hid
#    (hid duplicated per s-pair in SBUF), which also kills the separate
#    eh prologue and its Identity activation; b1 rides the tanh bias port.
#  - tanh on ScalarE in [128, 4*256] PSUM->SBUF bf16 tiles (quad of s).
#  - spatial sums via per-s DVE tensor_scalar(mult 1.0, accum_out=...) from
#    the bf16 tanh tile: SBUF-only 2-byte operands hit the 4x DVE mode
#    (tensor_reduce has no fast modes and would cost 2x more than tanh).
#  - context path entirely in fp8 to halve the dominant HBM traffic (the
#    measured wall tracks staged bytes at the ~210-250 GB/s/core the 8
#    phase-aligned cores actually get): enc is staged as e4m3 main + e4m3
#    residual (together ~13-bit, bf16-grade); attention weights as e4m3
#    sids + e5m2 residual sids (e5m2 normals reach 6e-5, so the ~0.002
#    residuals need no rescale and all three DoubleRow passes accumulate
#    into one PSUM group per c-chunk).
#  - the two batches are software-pipelined: batch 1's energy matmuls run
#    on PE while batch 0's softmax tail resolves on ACT/DVE, and batch 0's
#    context hides batch 1's softmax tail.
#  - numerics validated against the fp32 reference in numpy and on HW:
#    rel err 2.24e-3 (threshold 2e-2); energies are tiny (|e| < 0.13) so
#    exp without max-subtraction is safe. gpsimd is avoided for the sid
#    ops: real Q7 launch overhead (~2us/instr) is 6x the cost model's.
#
# Cost-model engine busy per rep: ACT ~34us, DVE ~39, PE ~26, DMA ~26
# (8.6 MiB staged); staged bytes dominate the HW number.

import numpy as np

B, S, C, A, HW = 16, 32, 256, 256, 256
NCORES = 8
BPC = B // NCORES  # batches per core


def build_program(reps: int = 1):
    import concourse.bacc as bacc
    import concourse.tile as tile
    from concourse import mybir
    from contextlib import ExitStack

    f32 = mybir.dt.float32
    f32r = mybir.dt.float32r
    bf16 = mybir.dt.bfloat16
    fp8 = mybir.dt.float8e4
    fp8e5 = mybir.dt.float8e5
    AF = mybir.ActivationFunctionType
    DR = mybir.MatmulPerfMode.DoubleRow
    ts = lambda i, n: slice(i * n, (i + 1) * n)

    nc = bacc.Bacc(
        "TRN2",
        target_bir_lowering=False,
        debug=False,
        enable_asserts=False,
        num_devices=NCORES,
    )

    # HBM tensors. enc8/encb/hid8 are host-packed as [b, p, kc, s, hw] with
    # c = kc*128 + p so each DMA lands directly in matmul-ready layout with
    # >=2KB contiguous runs per partition.
    enc8_d = nc.dram_tensor("enc8", [BPC, 128, 2, S, HW], fp8, kind="ExternalInput").ap()
    encr_d = nc.dram_tensor("encr", [BPC, 128, 2, S, HW], fp8, kind="ExternalInput").ap()
    hid8_d = nc.dram_tensor("hid8", [BPC, 128, 2, 2, HW], fp8, kind="ExternalInput").ap()
    w1e8_d = nc.dram_tensor("w1e8", [128, 2, A], fp8, kind="ExternalInput").ap()
    w1h8_d = nc.dram_tensor("w1h8", [128, 2, A], fp8, kind="ExternalInput").ap()
    b1_d = nc.dram_tensor("b1c", [128, A // 128], f32, kind="ExternalInput").ap()
    w2_d = nc.dram_tensor("w2c", [128, A // 128], f32r, kind="ExternalInput").ap()
    idb_d = nc.dram_tensor("identb", [128, 128], bf16, kind="ExternalInput").ap()
    ones_d = nc.dram_tensor("ones", [1, 128], f32r, kind="ExternalInput").ap()
    out_d = nc.dram_tensor("ctx", [BPC, C, HW], f32r, kind="ExternalOutput").ap()

    with tile.TileContext(nc) as tc, ExitStack() as ctx:
        consts = ctx.enter_context(tc.tile_pool(name="consts", bufs=1))
        encp8 = ctx.enter_context(tc.tile_pool(name="encp8", bufs=3))
        encpb = ctx.enter_context(tc.tile_pool(name="encpb", bufs=3))
        hidp = ctx.enter_context(tc.tile_pool(name="hidp", bufs=2))
        accp = ctx.enter_context(tc.tile_pool(name="accp", bufs=4))
        scrp = ctx.enter_context(tc.tile_pool(name="scrp", bufs=4))
        junkp = ctx.enter_context(tc.tile_pool(name="junkp", bufs=4))
        smp = ctx.enter_context(tc.tile_pool(name="smp", bufs=4))
        sidp = ctx.enter_context(tc.tile_pool(name="sidp", bufs=2))
        outp = ctx.enter_context(tc.tile_pool(name="outp", bufs=4))
        preps = ctx.enter_context(tc.tile_pool(name="preps", bufs=2, space="PSUM"))
        smallps = ctx.enter_context(tc.tile_pool(name="smallps", bufs=2, space="PSUM"))
        ctxps = ctx.enter_context(tc.tile_pool(name="ctxps", bufs=2, space="PSUM"))

        # constants
        w1e8_sb = consts.tile([128, 2, A], fp8, tag="w1e8_sb")
        nc.sync.dma_start(w1e8_sb[:], w1e8_d[:])
        w1h8_sb = consts.tile([128, 2, A], fp8, tag="w1h8_sb")
        nc.sync.dma_start(w1h8_sb[:], w1h8_d[:])
        b1_sb = consts.tile([128, 2], f32, tag="b1_sb")
        nc.sync.dma_start(b1_sb[:], b1_d[:])
        w2_sb = consts.tile([128, 2], f32r, tag="w2_sb")
        nc.sync.dma_start(w2_sb[:], w2_d[:])
        idb_sb = consts.tile([128, 128], bf16, tag="idb_sb")
        nc.sync.dma_start(idb_sb[:], idb_d[:])
        ones_sb = consts.tile([1, 128], f32r, tag="ones_sb")
        nc.sync.dma_start(ones_sb[:], ones_d[:])

        def phase_energy(b, hid8_sb, e8_t):
            """DR matmuls + tanh + 4x-mode accumulate; returns acc tiles."""
            acc = []
            for ach in range(2):
                a_t = accp.tile([128, S], f32r, tag="acc")
                acc.append(a_t)
            for ach in range(2):
                for quad in range(8):
                    pre = preps.tile([128, 4, HW], f32, tag="pre")
                    for half in range(2):
                        s0 = 4 * quad + 2 * half
                        nc.tensor.matmul(
                            pre[:, 2 * half : 2 * half + 2, :],
                            lhsT=w1e8_sb[:, :, ts(ach, 128)],
                            rhs=e8_t[:, :, s0 : s0 + 2, :],
                            start=True,
                            stop=False,
                            perf_mode=DR,
                        )
                        nc.tensor.matmul(
                            pre[:, 2 * half : 2 * half + 2, :],
                            lhsT=w1h8_sb[:, :, ts(ach, 128)],
                            rhs=hid8_sb[:],
                            start=False,
                            stop=True,
                            perf_mode=DR,
                        )
                    tq = scrp.tile([128, 4, HW], bf16, tag="tq")
                    nc.scalar.activation(
                        tq[:], pre[:], AF.Tanh, bias=b1_sb[:, ach : ach + 1]
                    )
                    with nc.allow_low_precision(reason="bf16 scratch, f32 accum"):
                        for si in range(4):
                            s = 4 * quad + si
                            scr = junkp.tile([128, HW], bf16, tag="scr")
                            nc.vector.tensor_scalar(
                                scr[:],
                                tq[:, si, :],
                                1.0,
                                0.0,
                                op0=mybir.AluOpType.mult,
                                op1=mybir.AluOpType.add,
                                accum_out=acc[ach][:, s : s + 1],
                            )
            return acc

        def phase_softmax(acc):
            """energies -> softmax -> fp8 sid pair (main + x16 residual)."""
            ep = smallps.tile([1, S], f32, tag="ep", bufs=1)
            for ach in range(2):
                nc.tensor.matmul(
                    ep[:],
                    lhsT=w2_sb[:, ach : ach + 1],
                    rhs=acc[ach][:],
                    start=(ach == 0),
                    stop=(ach == 1),
                )
            # softmax over S (|energy| < 0.2, exp safe without max-sub)
            esb = smp.tile([1, S], f32, tag="esb")
            nc.scalar.activation(esb[:], ep[:], AF.Exp)
            zsum = smp.tile([1, 1], f32, tag="zsum")
            nc.vector.reduce_sum(zsum[:], esb[:], axis=mybir.AxisListType.X)
            rz = smp.tile([1, 1], f32, tag="rz")
            nc.vector.reciprocal(rz[:], zsum[:])
            # ab2[0,0,:] = attn; ab2[0,1,:] = attn - fp8(attn). The residual
            # sids are stored e5m2 (normals reach 6e-5) so no rescale needed
            # and all three context passes share one PSUM accumulation group.
            ab2 = smp.tile([1, 2, S], f32r, tag="ab2")
            with nc.allow_low_precision(reason="fp8 sid chain, validated 2e-3"):
                nc.vector.tensor_scalar_mul(ab2[:, 0, :], esb[:], rz[:, 0:1])
                a8 = smp.tile([1, S], fp8, tag="a8")
                nc.vector.tensor_copy(a8[:], ab2[:, 0, :])
                nc.vector.scalar_tensor_tensor(
                    ab2[:, 1, :],
                    in0=a8[:],
                    scalar=-1.0,
                    in1=ab2[:, 0, :],
                    op0=mybir.AluOpType.mult,
                    op1=mybir.AluOpType.add,
                )
            # broadcast both rows to all 128 partitions with one K=1 matmul
            abc_ps = smallps.tile([128, 2, S], f32, tag="abc", bufs=1)
            nc.tensor.matmul(
                abc_ps[:], lhsT=ones_sb[:], rhs=ab2[:], start=True, stop=True
            )
            abc = smp.tile([128, 2, S], f32, tag="abc_sb")
            nc.vector.tensor_copy(abc[:], abc_ps[:])
            sid8 = sidp.tile([128, S, 128], fp8, tag="sid8")
            sidf8 = sidp.tile([128, S, 128], fp8e5, tag="sidf8")
            with nc.allow_low_precision(reason="fp8 sids feed fp8 DR matmul"):
                for s in range(S):
                    nc.vector.tensor_scalar_mul(
                        sid8[:, s, :], idb_sb[:], abc[:, 0, s : s + 1]
                    )
                for s in range(S):
                    nc.vector.tensor_scalar_mul(
                        sidf8[:, s, :], idb_sb[:], abc[:, 1, s : s + 1]
                    )
            return sid8, sidf8

        def phase_context(b, sids, e8_t, er_t):
            """DoubleRow fp8 context: a8*(enc8+r8) + af5*enc8, one PSUM
            accumulation group per c-chunk."""
            sid8, sidf8 = sids
            ctxp = []
            for kc in range(2):
                cp_t = ctxps.tile([128, HW], f32, tag="ctxp")
                ctxp.append(cp_t)
            for p in range(S // 2):
                s0 = 2 * p
                for kc in range(2):
                    nc.tensor.matmul(
                        ctxp[kc][:],
                        lhsT=sid8[:, s0 : s0 + 2, :],
                        rhs=e8_t[:, kc, s0 : s0 + 2, :],
                        start=(p == 0),
                        stop=False,
                        perf_mode=DR,
                    )
            for p in range(S // 2):
                s0 = 2 * p
                for kc in range(2):
                    nc.tensor.matmul(
                        ctxp[kc][:],
                        lhsT=sid8[:, s0 : s0 + 2, :],
                        rhs=er_t[:, kc, s0 : s0 + 2, :],
                        start=False,
                        stop=False,
                        perf_mode=DR,
                    )
            for p in range(S // 2):
                s0 = 2 * p
                for kc in range(2):
                    nc.tensor.matmul(
                        ctxp[kc][:],
                        lhsT=sidf8[:, s0 : s0 + 2, :],
                        rhs=e8_t[:, kc, s0 : s0 + 2, :],
                        start=False,
                        stop=(p == S // 2 - 1),
                        perf_mode=DR,
                    )
            for kc in range(2):
                osb = outp.tile([128, HW], f32r, tag="osb")
                nc.vector.tensor_copy(osb[:], ctxp[kc][:])
                nc.sync.dma_start(out_d[b, ts(kc, 128), :], osb[:])

        for rep in range(reps):
            # hidden first: tiny DMAs must not queue behind encoder traffic
            hid8_sb, e8_sb, eb_sb = [], [], []
            for b in range(BPC):
                h_t = hidp.tile([128, 2, 2, HW], fp8, tag="hid8_sb")
                nc.sync.dma_start(h_t[:], hid8_d[b])
                hid8_sb.append(h_t)
            for b in range(BPC):
                # encoder tiles: fp8 main for the energy path (needed first)
                # plus fp8 residual for the context path (needed after
                # softmax); single whole-tile DMAs keep the SP sequencer cheap
                e8_t = encp8.tile([128, 2, S, HW], fp8, tag="enc8_sb")
                nc.sync.dma_start(e8_t[:], enc8_d[b])
                e8_sb.append(e8_t)
                er_t = encpb.tile([128, 2, S, HW], fp8, tag="encr_sb")
                nc.sync.dma_start(er_t[:], encr_d[b])
                eb_sb.append(er_t)

            # interleave the two batches so batch 1's energy matmuls keep PE
            # busy while batch 0's softmax tail resolves on ACT/DVE, and
            # batch 0's context hides batch 1's softmax tail
            acc0 = phase_energy(0, hid8_sb[0], e8_sb[0])
            sid0 = phase_softmax(acc0)
            acc1 = phase_energy(1, hid8_sb[1], e8_sb[1])
            sid1 = phase_softmax(acc1)
            phase_context(0, sid0, e8_sb[0], eb_sb[0])
            phase_context(1, sid1, e8_sb[1], eb_sb[1])

    nc.compile()
    return nc


def make_in_maps(hidden_state, encoder_outputs, W1, b1, W2):
    import ml_dtypes

    e4 = ml_dtypes.float8_e4m3
    bf = ml_dtypes.bfloat16

    hs = np.ascontiguousarray(hidden_state, dtype=np.float32).reshape(B, C, HW)
    enc = np.ascontiguousarray(encoder_outputs, dtype=np.float32).reshape(B, S, C, HW)
    # [b, s, (kc p), hw] -> [b, p, kc, s, hw]
    enc_pack = enc.reshape(B, S, 2, 128, HW).transpose(0, 3, 2, 1, 4)
    enc8 = np.ascontiguousarray(enc_pack.astype(e4))
    encr = np.ascontiguousarray(
        (enc_pack - enc8.astype(np.float32)).astype(e4)
    )
    # hid [b, (kc p), hw] -> [b, p, kc, hw], duplicated over the s-pair dim
    hid_pack = hs.reshape(B, 2, 128, HW).transpose(0, 2, 1, 3)
    hid8 = np.ascontiguousarray(
        np.broadcast_to(hid_pack[:, :, :, None, :], (B, 128, 2, 2, HW)).astype(e4)
    )
    w1 = np.asarray(W1, np.float32)
    # W1 is [a, c]; lhsT layout [c_part, kc, a]
    w1e8 = np.ascontiguousarray(w1[:, C:].T.reshape(2, 128, A).transpose(1, 0, 2).astype(e4))
    w1h8 = np.ascontiguousarray(w1[:, :C].T.reshape(2, 128, A).transpose(1, 0, 2).astype(e4))
    b1c = np.ascontiguousarray(np.asarray(b1, np.float32).reshape(2, 128).T)
    w2c = np.ascontiguousarray((np.asarray(W2, np.float32)[0] / HW).reshape(2, 128).T)
    identb = np.eye(128, dtype=np.float32).astype(bf)
    ones = np.ones((1, 128), dtype=np.float32)
    in_maps = []
    for i in range(NCORES):
        in_maps.append(
            {
                "enc8": enc8[BPC * i : BPC * (i + 1)],
                "encr": encr[BPC * i : BPC * (i + 1)],
                "hid8": hid8[BPC * i : BPC * (i + 1)],
                "w1e8": w1e8,
                "w1h8": w1h8,
                "b1c": b1c,
                "w2c": w2c,
                "identb": identb,
                "ones": ones,
            }
        )
    return in_maps


def _wait_devices_healthy(max_tries=20, sleep_s=20):
    import time
    import jax

    for i in range(max_tries):
        try:
            for d in jax.devices()[:NCORES]:
                np.asarray(jax.device_put(np.ones(4, np.float32), d) + 1)
            return
        except Exception:
            if i == max_tries - 1:
                raise
            time.sleep(sleep_s)


def kernel(hidden_state, encoder_outputs, W1, b1, W2, b2, _profile=[None]):
    import os
    import time

    # The axon NTFF-profiling hook is unavailable in this environment; make
    # sure run_bass_kernel_spmd never takes the trace path.
    os.environ["BASS_NEVER_TRACE"] = "1"
    from concourse.bass_utils import run_bass_kernel_spmd

    _wait_devices_healthy()
    nc = build_program()
    in_maps = make_in_maps(hidden_state, encoder_outputs, W1, b1, W2)
    res = None
    for attempt in range(3):
        try:
            res = run_bass_kernel_spmd(nc, in_maps, list(range(NCORES)))
            break
        except Exception:
            if attempt == 2:
                raise
            time.sleep(30)
            _wait_devices_healthy()
    _profile[0] = res
    out = np.empty((B, C, 16, 16), dtype=np.float32)
    for i in range(NCORES):
        out[BPC * i : BPC * (i + 1)] = res.results[i]["ctx"].reshape(BPC, C, 16, 16)
    return out


# revision 20
# speedup vs baseline: 2.7332x; 1.2673x over previous
# Trainium2 Bass kernel for nn_AttentionBlock (additive attention over S):
#   pre = W1h @ hid (bcast over S) + W1e @ enc + b1      [B,S,A,HW]
#   energy[b,s] = mean_hw( W2 . tanh(pre) )              (+b2 dropped: softmax
#   attn = softmax_S(energy); ctx = sum_s attn[s]*enc     shift-invariant)
#
# Sharding: data-parallel over batch. B=16 over 8 cores -> 2 batches/core.
#
# v2 design (vs the 70.7us f32r baseline):
#  - energy path in fp8-e4m3 with DoubleRow matmuls: one instruction contracts
#    both 128-deep c-chunks at 0.5 cycles/row -> 4x less PE time than f32r.
#    The eh term is injected as a second DoubleRow matmul of W1h @ hid
#    (hid duplicated per s-pair in SBUF), which also kills the separate
#    eh prologue and its Identity activation; b1 rides the tanh bias port.
#  - tanh on ScalarE in [128, 4*256] PSUM->SBUF bf16 tiles (quad of s).
#  - spatial sums via per-s DVE tensor_scalar(mult 1.0, accum_out=...) from
#    the bf16 tanh tile: SBUF-only 2-byte operands hit the 4x DVE mode
#    (tensor_reduce has no fast modes and would cost 2x more than tanh).
#  - context path in bf16: host-cast enc copy, scaled-identity matmuls with
#    bf16 sids (built by 4x-mode tensor_scalar from the broadcast attn).
#  - fp8/bf16 numerics validated against the fp32 reference in numpy:
#    rel err 3.3e-3 (threshold 2e-2); energies are tiny (|e| < 0.13) so exp
#    without max-subtraction is safe.
#
# Projected engine busy per rep (cost model): DMA ~37-39us (12.5 MiB staged
# at ~345 GB/s), ACT ~34, PE ~28, DVE ~25.

import numpy as np

B, S, C, A, HW = 16, 32, 256, 256, 256
NCORES = 8
BPC = B // NCORES  # batches per core


def build_program(reps: int = 1):
    import concourse.bacc as bacc
    import concourse.tile as tile
    from concourse import mybir
    from contextlib import ExitStack

    f32 = mybir.dt.float32
    f32r = mybir.dt.float32r
    bf16 = mybir.dt.bfloat16
    fp8 = mybir.dt.float8e4
    AF = mybir.ActivationFunctionType
    DR = mybir.MatmulPerfMode.DoubleRow
    ts = lambda i, n: slice(i * n, (i + 1) * n)

    nc = bacc.Bacc(
        "TRN2",
        target_bir_lowering=False,
        debug=False,
        enable_asserts=False,
        num_devices=NCORES,
    )

    # HBM tensors. enc8/encb/hid8 are host-packed as [b, p, kc, s, hw] with
    # c = kc*128 + p so each DMA lands directly in matmul-ready layout with
    # >=2KB contiguous runs per partition.
    enc8_d = nc.dram_tensor("enc8", [BPC, 128, 2, S, HW], fp8, kind="ExternalInput").ap()
    encb_d = nc.dram_tensor("encb", [BPC, 128, 2, S, HW], bf16, kind="ExternalInput").ap()
    hid8_d = nc.dram_tensor("hid8", [BPC, 128, 2, 2, HW], fp8, kind="ExternalInput").ap()
    w1e8_d = nc.dram_tensor("w1e8", [128, 2, A], fp8, kind="ExternalInput").ap()
    w1h8_d = nc.dram_tensor("w1h8", [128, 2, A], fp8, kind="ExternalInput").ap()
    b1_d = nc.dram_tensor("b1c", [128, A // 128], f32, kind="ExternalInput").ap()
    w2_d = nc.dram_tensor("w2c", [128, A // 128], f32r, kind="ExternalInput").ap()
    idb_d = nc.dram_tensor("identb", [128, 128], bf16, kind="ExternalInput").ap()
    ones_d = nc.dram_tensor("ones", [1, 128], f32r, kind="ExternalInput").ap()
    out_d = nc.dram_tensor("ctx", [BPC, C, HW], f32r, kind="ExternalOutput").ap()

    with tile.TileContext(nc) as tc, ExitStack() as ctx:
        consts = ctx.enter_context(tc.tile_pool(name="consts", bufs=1))
        encp8 = ctx.enter_context(tc.tile_pool(name="encp8", bufs=3))
        encpb = ctx.enter_context(tc.tile_pool(name="encpb", bufs=3))
        hidp = ctx.enter_context(tc.tile_pool(name="hidp", bufs=2))
        accp = ctx.enter_context(tc.tile_pool(name="accp", bufs=4))
        scrp = ctx.enter_context(tc.tile_pool(name="scrp", bufs=4))
        junkp = ctx.enter_context(tc.tile_pool(name="junkp", bufs=4))
        smp = ctx.enter_context(tc.tile_pool(name="smp", bufs=4))
        sidp = ctx.enter_context(tc.tile_pool(name="sidp", bufs=2))
        outp = ctx.enter_context(tc.tile_pool(name="outp", bufs=4))
        preps = ctx.enter_context(tc.tile_pool(name="preps", bufs=2, space="PSUM"))
        smallps = ctx.enter_context(tc.tile_pool(name="smallps", bufs=2, space="PSUM"))
        ctxps = ctx.enter_context(tc.tile_pool(name="ctxps", bufs=2, space="PSUM"))

        # constants
        w1e8_sb = consts.tile([128, 2, A], fp8, tag="w1e8_sb")
        nc.sync.dma_start(w1e8_sb[:], w1e8_d[:])
        w1h8_sb = consts.tile([128, 2, A], fp8, tag="w1h8_sb")
        nc.sync.dma_start(w1h8_sb[:], w1h8_d[:])
        b1_sb = consts.tile([128, 2], f32, tag="b1_sb")
        nc.sync.dma_start(b1_sb[:], b1_d[:])
        w2_sb = consts.tile([128, 2], f32r, tag="w2_sb")
        nc.sync.dma_start(w2_sb[:], w2_d[:])
        idb_sb = consts.tile([128, 128], bf16, tag="idb_sb")
        nc.sync.dma_start(idb_sb[:], idb_d[:])
        ones_sb = consts.tile([1, 128], f32r, tag="ones_sb")
        nc.sync.dma_start(ones_sb[:], ones_d[:])

        def phase_energy(b, hid8_sb, e8_t):
            """DR matmuls + tanh + 4x-mode accumulate; returns acc tiles."""
            acc = []
            for ach in range(2):
                a_t = accp.tile([128, S], f32r, tag="acc")
                acc.append(a_t)
            for ach in range(2):
                for quad in range(8):
                    pre = preps.tile([128, 4, HW], f32, tag="pre")
                    for half in range(2):
                        s0 = 4 * quad + 2 * half
                        nc.tensor.matmul(
                            pre[:, 2 * half : 2 * half + 2, :],
                            lhsT=w1e8_sb[:, :, ts(ach, 128)],
                            rhs=e8_t[:, :, s0 : s0 + 2, :],
                            start=True,
                            stop=False,
                            perf_mode=DR,
                        )
                        nc.tensor.matmul(
                            pre[:, 2 * half : 2 * half + 2, :],
                            lhsT=w1h8_sb[:, :, ts(ach, 128)],
                            rhs=hid8_sb[:],
                            start=False,
                            stop=True,
                            perf_mode=DR,
                        )
                    tq = scrp.tile([128, 4, HW], bf16, tag="tq")
                    nc.scalar.activation(
                        tq[:], pre[:], AF.Tanh, bias=b1_sb[:, ach : ach + 1]
                    )
                    with nc.allow_low_precision(reason="bf16 scratch, f32 accum"):
                        for si in range(4):
                            s = 4 * quad + si
                            scr = junkp.tile([128, HW], bf16, tag="scr")
                            nc.vector.tensor_scalar(
                                scr[:],
                                tq[:, si, :],
                                1.0,
                                0.0,
                                op0=mybir.AluOpType.mult,
                                op1=mybir.AluOpType.add,
                                accum_out=acc[ach][:, s : s + 1],
                            )
            return acc

        def phase_softmax(acc):
            """energies -> softmax -> broadcast -> bf16 sids; returns sids."""
            ep = smallps.tile([1, S], f32, tag="ep", bufs=1)
            for ach in range(2):
                nc.tensor.matmul(
                    ep[:],
                    lhsT=w2_sb[:, ach : ach + 1],
                    rhs=acc[ach][:],
                    start=(ach == 0),
                    stop=(ach == 1),
                )
            # softmax over S (|energy| < 0.2, exp safe without max-sub)
            esb = smp.tile([1, S], f32, tag="esb")
            nc.scalar.activation(esb[:], ep[:], AF.Exp)
            zsum = smp.tile([1, 1], f32, tag="zsum")
            nc.vector.reduce_sum(zsum[:], esb[:], axis=mybir.AxisListType.X)
            rz = smp.tile([1, 1], f32, tag="rz")
            nc.vector.reciprocal(rz[:], zsum[:])
            attn = smp.tile([1, S], f32r, tag="attn")
            nc.vector.tensor_scalar_mul(attn[:], esb[:], rz[:, 0:1])
            # broadcast attn to all 128 partitions with a K=1 matmul
            abc_ps = smallps.tile([128, S], f32, tag="abc", bufs=1)
            nc.tensor.matmul(
                abc_ps[:], lhsT=ones_sb[:], rhs=attn[:], start=True, stop=True
            )
            abc = smp.tile([128, S], f32, tag="abc_sb")
            nc.vector.tensor_copy(abc[:], abc_ps[:])
            sid_all = sidp.tile([128, S, 128], bf16, tag="sid_all")
            with nc.allow_low_precision(reason="bf16 sids feed bf16 matmul"):
                for s in range(S):
                    nc.vector.tensor_scalar_mul(
                        sid_all[:, s, :], idb_sb[:], abc[:, s : s + 1]
                    )
            return sid_all

        def phase_context(b, sid_all, eb_t):
            """per-s scaled-identity matmuls over the bf16 enc tile + store."""
            ctxp = []
            for kc in range(2):
                c_t = ctxps.tile([128, HW], f32, tag="ctxp")
                ctxp.append(c_t)
            for s in range(S):
                for kc in range(2):
                    nc.tensor.matmul(
                        ctxp[kc][:],
                        lhsT=sid_all[:, s, :],
                        rhs=eb_t[:, kc, s, :],
                        start=(s == 0),
                        stop=(s == S - 1),
                    )
            for kc in range(2):
                osb = outp.tile([128, HW], f32r, tag="osb")
                nc.vector.tensor_copy(osb[:], ctxp[kc][:])
                nc.sync.dma_start(out_d[b, ts(kc, 128), :], osb[:])

        for rep in range(reps):
            # hidden first: tiny DMAs must not queue behind encoder traffic
            hid8_sb, e8_sb, eb_sb = [], [], []
            for b in range(BPC):
                h_t = hidp.tile([128, 2, 2, HW], fp8, tag="hid8_sb")
                nc.sync.dma_start(h_t[:], hid8_d[b])
                hid8_sb.append(h_t)
            for b in range(BPC):
                # encoder tiles: fp8 for the energy path (needed first),
                # bf16 for the context path (needed only after softmax);
                # single whole-tile DMAs keep the SP sequencer cheap
                e8_t = encp8.tile([128, 2, S, HW], fp8, tag="enc8_sb")
                nc.sync.dma_start(e8_t[:], enc8_d[b])
                e8_sb.append(e8_t)
                eb_t = encpb.tile([128, 2, S, HW], bf16, tag="encb_sb")
                nc.sync.dma_start(eb_t[:], encb_d[b])
                eb_sb.append(eb_t)

            # interleave the two batches so batch 1's energy matmuls keep PE
            # busy while batch 0's softmax tail resolves on ACT/DVE, and
            # batch 0's context hides batch 1's softmax tail
            acc0 = phase_energy(0, hid8_sb[0], e8_sb[0])
            sid0 = phase_softmax(acc0)
            acc1 = phase_energy(1, hid8_sb[1], e8_sb[1])
            sid1 = phase_softmax(acc1)
            phase_context(0, sid0, eb_sb[0])
            phase_context(1, sid1, eb_sb[1])

    nc.compile()
    return nc


def make_in_maps(hidden_state, encoder_outputs, W1, b1, W2):
    import ml_dtypes

    e4 = ml_dtypes.float8_e4m3
    bf = ml_dtypes.bfloat16

    hs = np.ascontiguousarray(hidden_state, dtype=np.float32).reshape(B, C, HW)
    enc = np.ascontiguousarray(encoder_outputs, dtype=np.float32).reshape(B, S, C, HW)
    # [b, s, (kc p), hw] -> [b, p, kc, s, hw]
    enc_pack = enc.reshape(B, S, 2, 128, HW).transpose(0, 3, 2, 1, 4)
    enc8 = np.ascontiguousarray(enc_pack.astype(e4))
    encb = np.ascontiguousarray(enc_pack.astype(bf))
    # hid [b, (kc p), hw] -> [b, p, kc, hw], duplicated over the s-pair dim
    hid_pack = hs.reshape(B, 2, 128, HW).transpose(0, 2, 1, 3)
    hid8 = np.ascontiguousarray(
        np.broadcast_to(hid_pack[:, :, :, None, :], (B, 128, 2, 2, HW)).astype(e4)
    )
    w1 = np.asarray(W1, np.float32)
    # W1 is [a, c]; lhsT layout [c_part, kc, a]
    w1e8 = np.ascontiguousarray(w1[:, C:].T.reshape(2, 128, A).transpose(1, 0, 2).astype(e4))
    w1h8 = np.ascontiguousarray(w1[:, :C].T.reshape(2, 128, A).transpose(1, 0, 2).astype(e4))
    b1c = np.ascontiguousarray(np.asarray(b1, np.float32).reshape(2, 128).T)
    w2c = np.ascontiguousarray((np.asarray(W2, np.float32)[0] / HW).reshape(2, 128).T)
    identb = np.eye(128, dtype=np.float32).astype(bf)
    ones = np.ones((1, 128), dtype=np.float32)
    in_maps = []
    for i in range(NCORES):
        in_maps.append(
            {
                "enc8": enc8[BPC * i : BPC * (i + 1)],
                "encb": encb[BPC * i : BPC * (i + 1)],
                "hid8": hid8[BPC * i : BPC * (i + 1)],
                "w1e8": w1e8,
                "w1h8": w1h8,
                "b1c": b1c,
                "w2c": w2c,
                "identb": identb,
                "ones": ones,
            }
        )
    return in_maps


def _wait_devices_healthy(max_tries=20, sleep_s=20):
    import time
    import jax

    for i in range(max_tries):
        try:
            for d in jax.devices()[:NCORES]:
                np.asarray(jax.device_put(np.ones(4, np.float32), d) + 1)
            return
        except Exception:
            if i == max_tries - 1:
                raise
            time.sleep(sleep_s)


def kernel(hidden_state, encoder_outputs, W1, b1, W2, b2, _profile=[None]):
    import os
    import time

    # The axon NTFF-profiling hook is unavailable in this environment; make
    # sure run_bass_kernel_spmd never takes the trace path.
    os.environ["BASS_NEVER_TRACE"] = "1"
    from concourse.bass_utils import run_bass_kernel_spmd

    _wait_devices_healthy()
    nc = build_program()
    in_maps = make_in_maps(hidden_state, encoder_outputs, W1, b1, W2)
    res = None
    for attempt in range(3):
        try:
            res = run_bass_kernel_spmd(nc, in_maps, list(range(NCORES)))
            break
        except Exception:
            if attempt == 2:
                raise
            time.sleep(30)
            _wait_devices_healthy()
    _profile[0] = res
    out = np.empty((B, C, 16, 16), dtype=np.float32)
    for i in range(NCORES):
        out[BPC * i : BPC * (i + 1)] = res.results[i]["ctx"].reshape(BPC, C, 16, 16)
    return out
